# revision 1
# baseline (speedup 1.0000x reference)
"""GraphTransformer refiner on 8 Trainium2 NeuronCores — v2.

Strategy (1D node-parallel, dst-sharded), revised from baseline:
- Host: shard dst nodes across 8 cores; per core sort local nodes by
  in-degree, tile 128; per-tile slot count padded to a multiple of 4,
  uniform across cores (one SPMD program). All weight matrices are
  pre-folded with W_in on host (f64) so the device never computes h:
    kv  = x @ (W_in[Wk|Wv]) + b_in[Wk|Wv]
    q   = x @ (W_in Wq)/sqrt(C) + (b_in Wq + bq)/sqrt(C)
    skip= x @ (W_in Wskip W_out);  bias folds into one output bias.
- Device: build bf16 [k|v] table for ALL nodes straight from x^T
  (bias via PSUM preload), ONE wide indirect DMA gather per dst tile
  (Dt*128 rows), whole-tile q.k scores on DVE, exp on the compact
  [128, Dt*4] score tensor, per-head exp*v, slot accumulation via
  identity matmul, denominators reduced from the compact exp tensor.
- Host: transpose, un-permute, concatenate.
"""

import numpy as np
import ml_dtypes

N, E, IN, HD, OUT, H, C = 50000, 800000, 128, 128, 32, 4, 32
NCORES = 8
SHARD = N // NCORES            # 6250
LT = 49                        # local node tiles (49*128 = 6272)
LPAD = LT * 128
NTAB = 98 * 512                # padded kv-table rows (50176)
DUMMY = N                      # zeroed dummy row for pad slots
BF16 = ml_dtypes.bfloat16


def _prep_edges(edge_index):
    src = np.asarray(edge_index[0], np.int64)
    dst = np.asarray(edge_index[1], np.int64)
    deg = np.bincount(dst, minlength=N)
    csr = np.zeros(N + 1, np.int64)
    np.cumsum(deg, out=csr[1:])
    order = np.argsort(dst, kind="stable")
    src_sorted = src[order].astype(np.int32)

    perms = []
    degs_sorted = np.zeros((NCORES, LPAD), np.int64)
    for c in range(NCORES):
        ldeg = deg[c * SHARD:(c + 1) * SHARD]
        perm = np.argsort(-ldeg, kind="stable")
        perms.append(perm)
        degs_sorted[c, :SHARD] = ldeg[perm]

    # uniform per-tile slot counts across cores, padded to multiple of 4
    tile_max = degs_sorted.reshape(NCORES, LT, 128).max(axis=(0, 2))
    D = np.maximum(4, ((tile_max + 3) // 4) * 4).astype(np.int64)
    offs = np.zeros(LT + 1, np.int64)
    np.cumsum(D, out=offs[1:])
    S_total = int(offs[-1])

    li = np.arange(SHARD)
    rows_all = li % 128
    tiles_all = li // 128
    idxs, padcs = [], []
    for c in range(NCORES):
        nodes = c * SHARD + perms[c]
        d = deg[nodes]
        Ec = int(d.sum())
        pref = np.zeros(SHARD, np.int64)
        np.cumsum(d[:-1], out=pref[1:])
        within = np.arange(Ec, dtype=np.int64) - np.repeat(pref, d)
        pos = np.repeat(csr[nodes], d) + within
        row_ids = np.repeat(rows_all, d)
        col_ids = offs[np.repeat(tiles_all, d)] + within
        idx = np.full((128, S_total), DUMMY, np.int32)
        idx[row_ids, col_ids] = src_sorted[pos]
        idxs.append(idx)
        dmat = degs_sorted[c].reshape(LT, 128).T       # [128, LT]
        padc = (D[None, :] - np.maximum(dmat, 1)).astype(np.float32)
        padcs.append(padc)
    return perms, D, offs, S_total, idxs, padcs


def _build(S_total, D, offs):
    import concourse.bacc as bacc
    import concourse.bass as bass
    import concourse.mybir as mybir
    import concourse.tile as tile
    from concourse.masks import make_identity
    from concourse.vector_clock import ScopedClock

    # this walrus build rejects >1 sync wait on a Drain; split the
    # kernel-tail drain into a chain of single-wait drains
    def _drain_split(self, tick_clock, wait_clock):
        drain_inst = self.nc.sync.drain()
        wait_clock.add_sem_waits(
            drain_inst.ins, ScopedClock({None: tick_clock.global_clock}))
        si = drain_inst.ins.sync_info
        if si is not None and len(si.on_wait) > 1:
            waits = list(si.on_wait)
            drain_inst.ins.sync_info = mybir.SyncInfo(
                on_wait=waits[:1], on_update=list(si.on_update))
            for w in waits[1:]:
                d2 = self.nc.sync.drain()
                d2.ins.sync_info = mybir.SyncInfo(on_wait=[w], on_update=[])
        self.nc.all_engine_barrier()
        popped = self.nc._tile_sem_poison_stack.pop()
        assert popped is self._sem_poison
        self.nc.clear_and_free_semaphores(list(self.sems.allocated().values()))
        self.nc.all_engine_barrier()

    tile.TileContext._drain_and_barrier = _drain_split

    F32, BF, I32 = mybir.dt.float32, mybir.dt.bfloat16, mybir.dt.int32
    MUL, ADD, SUB = (mybir.AluOpType.mult, mybir.AluOpType.add,
                     mybir.AluOpType.subtract)
    X = mybir.AxisListType.X
    Dmax = int(D.max())

    def view(ap, dims, extra_off=0):
        return bass.AP(ap.tensor, ap.offset + extra_off, [ap.ap[0]] + dims)

    nc = bacc.Bacc("TRN2", target_bir_lowering=False, debug=False,
                   num_devices=1, num_swdge_queues=4,
                   dynamic_dma_scratch_size=16384)
    xt_d = nc.dram_tensor("xt", [128, NTAB], BF, kind="ExternalInput").ap()
    xl_d = nc.dram_tensor("xl", [128, LPAD], BF, kind="ExternalInput").ap()
    idx_d = nc.dram_tensor("idx", [128, S_total], I32,
                           kind="ExternalInput").ap()
    padc_d = nc.dram_tensor("padc", [128, LT], F32, kind="ExternalInput").ap()
    wkv_d = nc.dram_tensor("wkv", [128, 256], BF, kind="ExternalInput").ap()
    wq_d = nc.dram_tensor("wq", [128, 128], BF, kind="ExternalInput").ap()
    bq_d = nc.dram_tensor("bq", [1, 128], BF, kind="ExternalInput").ap()
    wout_d = nc.dram_tensor("wout", [128, OUT], BF, kind="ExternalInput").ap()
    wsk_d = nc.dram_tensor("wsk", [128, OUT], BF, kind="ExternalInput").ap()
    bfin_d = nc.dram_tensor("bfin", [OUT, 1], F32, kind="ExternalInput").ap()
    tab_d = nc.dram_tensor("kvtab", [NTAB, 256], BF, kind="Internal").ap()
    out_d = nc.dram_tensor("outT", [OUT, LPAD], F32,
                           kind="ExternalOutput").ap()

    with tile.TileContext(nc) as tc:
        with (
            tc.tile_pool(name="sb", bufs=1) as sb,
            tc.tile_pool(name="sb3", bufs=3) as sb3,
            tc.tile_pool(name="kvp", bufs=3) as kvp,
            tc.tile_pool(name="wrk", bufs=2) as wrk,
            tc.tile_pool(name="ps", bufs=2, space="PSUM") as ps,
            tc.tile_pool(name="psA", bufs=2, space="PSUM") as psAp,
            tc.tile_pool(name="psB", bufs=2, space="PSUM") as psBp,
        ):
            # constants
            wkv = sb.tile([128, 256], BF, tag="wkv")
            wq = sb.tile([128, 128], BF, tag="wq")
            bq = sb.tile([1, 128], BF, tag="bq")
            wout = sb.tile([128, OUT], BF, tag="wout")
            wsk = sb.tile([128, OUT], BF, tag="wsk")
            bfin = sb.tile([OUT, 1], F32, tag="bfin")
            ones = sb.tile([1, 128], BF, tag="ones")
            ident = sb.tile([128, 128], BF, tag="ident")
            zrow = sb.tile([1, 256], BF, tag="zrow")
            idx = sb.tile([128, S_total], I32, tag="idx")
            padc = sb.tile([128, LT], F32, tag="padc")
            xl = sb.tile([128, LPAD], BF, tag="xl")
            qsb = sb.tile([128, LPAD], BF, tag="qsb")
            for t_, d_ in ((wkv, wkv_d), (wq, wq_d),
                           (bq, bq_d), (wout, wout_d), (wsk, wsk_d),
                           (bfin, bfin_d), (idx, idx_d), (padc, padc_d),
                           (xl, xl_d)):
                nc.sync.dma_start(t_[:], d_[:])
            nc.gpsimd.memset(ones[:], 1.0)
            nc.gpsimd.memset(zrow[:], 0.0)
            make_identity(nc, ident[:])

            # phase A: q for local nodes (permuted order), from xl directly
            for t in range(LT):
                s = slice(t * 128, (t + 1) * 128)
                qps = ps.tile([128, 512], mybir.dt.float32, tag="mm")
                nc.tensor.matmul(out=qps[:, :128], lhsT=xl[:, s], rhs=wq[:],
                                 start=True, stop=False)
                nc.tensor.matmul(out=qps[:, :128], lhsT=ones[:], rhs=bq[:],
                                 start=False, stop=True)
                nc.scalar.copy(qsb[:, s], qps[:, :128])

            # phase B: kv table (no bias: q.bk is per-dst constant so
            # softmax drops it; bv folds into bfin since alpha sums to 1).
            # PSUM->SBUF evacuation split across DVE and Act in parallel.
            for t in range(49):
                s = slice(t * 1024, (t + 1) * 1024)
                xb = sb3.tile([128, 1024], BF, tag="xb")
                nc.sync.dma_start(xb[:], xt_d[:, s])
                kvsb = sb3.tile([128, 2048], BF, tag="kvsb")
                for p2 in range(4):
                    kvps = psBp.tile([128, 512], mybir.dt.float32, tag="kvb")
                    for c2 in range(2):
                        c4 = p2 * 2 + c2
                        nc.tensor.matmul(
                            out=kvps[:, c2 * 256:(c2 + 1) * 256],
                            lhsT=xb[:, c4 * 128:(c4 + 1) * 128], rhs=wkv[:],
                            start=True, stop=True)
                    dst = kvsb[:, p2 * 512:(p2 + 1) * 512]
                    if p2 % 2 == 0:
                        nc.vector.tensor_copy(dst, kvps[:])
                    else:
                        nc.scalar.copy(dst, kvps[:])
                dst_ap = bass.AP(tab_d.tensor, t * 1024 * 256,
                                 [[256, 128], [128 * 256, 8], [1, 256]])
                nc.sync.dma_start(dst_ap, kvsb[:])
            nc.sync.dma_start(tab_d[DUMMY:DUMMY + 1, :], zrow[:])

            # phase C: edge attention per dst tile
            for t in range(LT):
                s = slice(t * 128, (t + 1) * 128)
                Dt, o = int(D[t]), int(offs[t])
                kvt = kvp.tile([128, Dmax * 256], BF, tag="kvt")
                for sj in range(Dt):
                    gin = nc.gpsimd.indirect_dma_start(
                        out=kvt[:, sj * 256:(sj + 1) * 256],
                        out_offset=None, in_=tab_d[:],
                        in_offset=bass.IndirectOffsetOnAxis(
                            ap=idx[:, o + sj:o + sj + 1], axis=0))
                    gin.ins.queue = f"qPoolDynamic{sj % 4 or ''}"

                # scores: smul = q (bcast over slots) * k
                smul = wrk.tile([128, Dmax * 128], BF, tag="smul")
                q_b = view(qsb[:, s], [[0, Dt], [1, 128]])
                nc.vector.tensor_tensor(
                    out=smul[:, :Dt * 128].rearrange("p (s f) -> p s f", s=Dt),
                    in0=q_b, in1=view(kvt[:], [[256, Dt], [1, 128]]), op=MUL)
                sc = sb3.tile([128, Dmax * 4], BF, tag="sc")
                with nc.allow_low_precision(reason="scores are O(0.1); bf16 "
                                            "rounding is ~1e-4 relative"):
                    nc.vector.tensor_reduce(
                        out=sc[:, :Dt * 4],
                        in_=smul[:, :Dt * 128].rearrange("p (g c) -> p g c",
                                                         c=C),
                        axis=X, op=ADD)
                # exp(score) broadcast over C, overwriting smul (tc WAR dep)
                sc_b = view(sc[:], [[1, Dt * 4], [0, C]])
                nc.scalar.activation(
                    out=smul[:, :Dt * 128].rearrange("p (g c) -> p g c", c=C),
                    in_=sc_b, func=mybir.ActivationFunctionType.Exp)

                # denominators from the c=0 plane of the broadcast exp
                den = sb3.tile([128, H], mybir.dt.float32, tag="den")
                nc.vector.tensor_reduce(
                    out=view(den[:], [[1, H], [0, 1]]),
                    in_=view(smul[:], [[C, H], [128, Dt]]), axis=X, op=ADD)

                # pv = exp(score) * v, whole tile, all-bf16 contiguous
                pv = wrk.tile([128, Dmax * 128], BF, tag="pv")
                nc.vector.tensor_tensor(
                    out=pv[:, :Dt * 128].rearrange("p (s f) -> p s f", s=Dt),
                    in0=smul[:, :Dt * 128].rearrange("p (s f) -> p s f", s=Dt),
                    in1=view(kvt[:], [[256, Dt], [1, 128]], 128), op=MUL)

                # accumulate pv over slot groups of 4 via identity matmul
                psA = psAp.tile([128, 512], mybir.dt.float32, tag="psA")
                ngr = Dt // 4
                for g in range(ngr):
                    nc.tensor.matmul(out=psA[:],
                                     lhsT=ident[:],
                                     rhs=pv[:, g * 512:(g + 1) * 512],
                                     start=(g == 0), stop=(g == ngr - 1))
                acc = sb3.tile([128, 128], mybir.dt.float32, tag="acc")
                nc.vector.tensor_reduce(
                    out=view(acc[:], [[1, 128], [0, 1]]),
                    in_=view(psA[:], [[1, 128], [128, 4]]), axis=X, op=ADD)

                den2 = sb3.tile([128, H], mybir.dt.float32, tag="den2")
                nc.vector.tensor_tensor(
                    out=den2[:], in0=den[:],
                    in1=view(padc[:, t:t + 1], [[0, H]]), op=SUB)
                rden = sb3.tile([128, H], mybir.dt.float32, tag="rden")
                nc.vector.reciprocal(rden[:], den2[:])
                zt = sb3.tile([128, 128], BF, tag="zt")
                nc.vector.tensor_tensor(
                    out=zt[:].rearrange("p (h c) -> p h c", h=H),
                    in0=acc[:].rearrange("p (h c) -> p h c", h=H),
                    in1=view(rden[:], [[1, H], [0, C]]), op=MUL)
                ztp = ps.tile([128, 128], BF, tag="ztp")
                nc.tensor.transpose(out=ztp[:], in_=zt[:], identity=ident[:])
                ztsb = sb3.tile([128, 128], BF, tag="ztsb")
                nc.scalar.copy(ztsb[:], ztp[:])
                opsT = ps.tile([128, 512], mybir.dt.float32, tag="mm")
                ops = opsT[0:OUT, 0:128]
                nc.tensor.matmul(out=ops, lhsT=wout[:], rhs=ztsb[:],
                                 start=True, stop=False)
                nc.tensor.matmul(out=ops, lhsT=wsk[:], rhs=xl[:, s],
                                 start=False, stop=True)
                osb = sb3.tile([OUT, 128], mybir.dt.float32, tag="osb")
                nc.scalar.activation(
                    out=osb[:], in_=ops,
                    func=mybir.ActivationFunctionType.Identity,
                    bias=bfin[:, 0:1])
                nc.sync.dma_start(out_d[:, s], osb[:])
    nc.compile()
    return nc


def _make_in_maps(inputs, x, perms, idxs, padcs):
    g = lambda k: np.asarray(inputs[k], np.float64)
    W_in, b_in, Wq, bq = g("W_in"), g("b_in"), g("Wq"), g("bq")
    Wk, Wv, bv = g("Wk"), g("Wv"), g("bv")
    Wskip, bskip, W_out, b_out = g("Wskip"), g("bskip"), g("W_out"), g("b_out")

    scale = 1.0 / np.sqrt(C)
    xt = np.zeros((128, NTAB), BF16)
    xt[:, :N] = x.T.astype(BF16)
    wkv_f = W_in @ np.concatenate([Wk, Wv], 1)            # [128, 256]
    wkv = wkv_f.astype(np.float32).astype(BF16)
    wq = ((W_in @ Wq) * scale).astype(np.float32).astype(BF16)
    bqs = ((b_in @ Wq + bq) * scale).reshape(1, HD).astype(
        np.float32).astype(BF16)
    woutb = W_out.astype(np.float32).astype(BF16)
    wskb = (W_in @ Wskip @ W_out).astype(np.float32).astype(BF16)
    bfin = (b_out + (bv + b_in @ Wv) @ W_out + bskip @ W_out
            + b_in @ Wskip @ W_out).reshape(OUT, 1).astype(np.float32)

    in_maps = []
    for c in range(NCORES):
        xl = np.zeros((128, LPAD), BF16)
        blk = x[c * SHARD:(c + 1) * SHARD][perms[c]]
        xl[:, :SHARD] = blk.T.astype(BF16)
        in_maps.append({
            "xt": xt, "xl": xl, "idx": idxs[c], "padc": padcs[c],
            "wkv": wkv, "wq": wq, "bq": bqs,
            "wout": woutb, "wsk": wskb, "bfin": bfin,
        })
    return in_maps


def kernel(x, edge_index, W_in, b_in, Wq, bq, Wk, bk, Wv, bv, Wskip, bskip,
           W_out, b_out):
    x = np.asarray(x, np.float32)
    perms, D, offs, S_total, idxs, padcs = _prep_edges(edge_index)
    inputs = dict(W_in=W_in, b_in=b_in, Wq=Wq, bq=bq, Wk=Wk, Wv=Wv, bv=bv,
                  Wskip=Wskip, bskip=bskip, W_out=W_out, b_out=b_out)
    in_maps = _make_in_maps(inputs, x, perms, idxs, padcs)

    nc = _build(S_total, D, offs)
    from concourse import bass_utils
    res = bass_utils.run_bass_kernel_spmd(nc, in_maps,
                                          core_ids=list(range(NCORES)))
    out = np.empty((N, OUT), np.float32)
    for c in range(NCORES):
        oT = res.results[c]["outT"]
        out[c * SHARD + perms[c]] = oT[:, :SHARD].T
    return out

